# revision 4
# baseline (speedup 1.0000x reference)
"""CompressedGPT2Attention on 8 TRN2 NeuronCores.

Sharding: core c = (batch b = c // 2, head-group g = c % 2) — data parallel on
B=4, tensor parallel over 16 heads (8 per group). Each core computes a partial
output [S, E] (its head-group's contribution, + output_bias on g==0 cores);
host sums the two partials per batch.

Per-core pipeline (all matmul operands float32r = raw fp32 bits, PE rounds):
  hs_t [E, S] (host-transposed)
  q^T, k^T = W^T @ hs_t   [512, S]  (heads pair-packed: 2 heads / 128 partitions)
  v_aug    = hs @ Wv_aug  [S, 8*33] (per head 32 v-cols + ones col -> denominator)
  per head, scores^T[j, i] = k_h^T.T @ q_h^T  (K=64), block-causal
  exp via ScalarE (scale=1/8 folded), triangular mask on diagonal blocks
  attn_aug^T[33, i] += v_aug_jc^T.T @ exp_jc   (PSUM accum over j-chunks)
  normalize rows 0..31 by reciprocal of row 32 (DMA partition-broadcast)
  out[i, e] = attn^T.T @ Wout (+ bias via K=1 ones matmul)
"""

import numpy as np
from contextlib import ExitStack

import concourse.bass as bass
import concourse.bacc as bacc
import concourse.tile as tile
import concourse.mybir as mybir
from concourse.bass_utils import run_bass_kernel_spmd

F32 = mybir.dt.float32
F32R = mybir.dt.float32r
AF = mybir.ActivationFunctionType

B, S, E = 4, 2048, 1024
H, HD, R = 16, 64, 32
HG = 8                # heads per core
N_CORES = 8
SCALE = 1.0 / 8.0     # 1/sqrt(HD)

_PROGRAM_CACHE = {}


def _build_program():
    nc = bacc.Bacc("TRN2", target_bir_lowering=False, debug=False,
                   num_devices=N_CORES)

    hs_t = nc.dram_tensor("hs_t", [E, S], F32R, kind="ExternalInput").ap()
    wq = nc.dram_tensor("wq", [E, 512], F32R, kind="ExternalInput").ap()
    wk = nc.dram_tensor("wk", [E, 512], F32R, kind="ExternalInput").ap()
    bq = nc.dram_tensor("bq", [1, 512], F32R, kind="ExternalInput").ap()
    bk = nc.dram_tensor("bk", [1, 512], F32R, kind="ExternalInput").ap()
    wv = nc.dram_tensor("wv", [E, HG * 33], F32R, kind="ExternalInput").ap()
    bv = nc.dram_tensor("bv", [1, HG * 33], F32R, kind="ExternalInput").ap()
    wout = nc.dram_tensor("wout", [256, E], F32R, kind="ExternalInput").ap()
    bout = nc.dram_tensor("bout", [1, E], F32R, kind="ExternalInput").ap()
    tri = nc.dram_tensor("tri", [128, 128], F32, kind="ExternalInput").ap()
    out = nc.dram_tensor("out", [S, E], F32, kind="ExternalOutput").ap()

    with tile.TileContext(nc) as tc, ExitStack() as ctx:
        persist = ctx.enter_context(tc.tile_pool(name="persist", bufs=1))

        # ---- constants ----
        ones_f = persist.tile([1, 512], F32, name="ones_f", tag="ones_f")
        nc.vector.memset(ones_f, 1.0)
        ones = persist.tile([1, 512], F32R, name="ones", tag="ones")
        nc.vector.tensor_copy(out=ones, in_=ones_f)
        tri_sb = persist.tile([128, 128], F32, name="tri", tag="tri")
        nc.sync.dma_start(out=tri_sb, in_=tri)

        # ---- persistent activations ----
        q_sb = [persist.tile([128, S], F32R, name=f"q{m}", tag=f"q{m}") for m in range(4)]
        k_sb = [persist.tile([128, S], F32R, name=f"k{m}", tag=f"k{m}") for m in range(4)]
        v_sb = [persist.tile([128, HG * 33], F32R, name=f"v{sc}", tag=f"v{sc}") for sc in range(16)]
        attn_sb = [persist.tile([128, S], F32R, name=f"attn{t}", tag=f"attn{t}") for t in range(2)]

        # =========== phase 1: projections ===========
        with ExitStack() as pctx:
            hs_pool = pctx.enter_context(tc.tile_pool(name="hs", bufs=1))
            w_pool = pctx.enter_context(tc.tile_pool(name="w", bufs=1))
            b_pool = pctx.enter_context(tc.tile_pool(name="b", bufs=1))
            pp = pctx.enter_context(tc.tile_pool(name="pp", bufs=4, space="PSUM"))

            hs_sb = []
            for ec in range(8):
                t = hs_pool.tile([128, S], F32R, name=f"hs{ec}", tag=f"hs{ec}")
                nc.sync.dma_start(out=t, in_=hs_t[ec * 128:(ec + 1) * 128, :])
                hs_sb.append(t)

            # q^T and k^T: psum[col 128, s 512] = sum_e w[e, col].T @ hs_t[e, s]
            for which, w_dram, b_dram, dst in ((0, wq, bq, q_sb), (1, wk, bk, k_sb)):
                w_sb = []
                for ec in range(8):
                    t = w_pool.tile([128, 512], F32R, name=f"w{ec}", tag=f"w{ec}")
                    nc.sync.dma_start(out=t, in_=w_dram[ec * 128:(ec + 1) * 128, :])
                    w_sb.append(t)
                b_sb = b_pool.tile([1, 512], F32R, name=f"bqk{which}", tag=f"bqk{which}")
                nc.sync.dma_start(out=b_sb, in_=b_dram)
                for m in range(4):
                    for nb in range(4):
                        ps = pp.tile([128, 512], F32, name="proj", tag="proj")
                        sl = slice(nb * 512, nb * 512 + 512)
                        for ec in range(8):
                            nc.tensor.matmul(ps, w_sb[ec][:, m * 128:(m + 1) * 128],
                                             hs_sb[ec][:, sl],
                                             start=(ec == 0), stop=False)
                        nc.tensor.matmul(ps, b_sb[:, m * 128:(m + 1) * 128],
                                         ones, start=False, stop=True)
                        nc.vector.tensor_copy(out=dst[m][:, sl], in_=ps)

            # v_aug: psum[s 128, 264] = sum_e hs_t[e, s].T @ wv[e, :]
            wv_sb = []
            for ec in range(8):
                t = w_pool.tile([128, HG * 33], F32R, name=f"wv{ec}", tag=f"wv{ec}")
                nc.sync.dma_start(out=t, in_=wv[ec * 128:(ec + 1) * 128, :])
                wv_sb.append(t)
            bv_sb = b_pool.tile([1, HG * 33], F32R, name="bv", tag="bv")
            nc.sync.dma_start(out=bv_sb, in_=bv)
            for sc in range(16):
                ps = pp.tile([128, HG * 33], F32, name="vproj", tag="vproj")
                for ec in range(8):
                    nc.tensor.matmul(ps, hs_sb[ec][:, sc * 128:(sc + 1) * 128],
                                     wv_sb[ec], start=(ec == 0), stop=False)
                nc.tensor.matmul(ps, ones[:, 0:128], bv_sb, start=False, stop=True)
                nc.vector.tensor_copy(out=v_sb[sc], in_=ps)

        # =========== phase 2: attention ===========
        with ExitStack() as actx:
            sc_pool = actx.enter_context(tc.tile_pool(name="scps", bufs=1, space="PSUM"))
            at_pool = actx.enter_context(tc.tile_pool(name="atps", bufs=1, space="PSUM"))
            exp_pool = actx.enter_context(tc.tile_pool(name="exp", bufs=4))
            nrm_pool = actx.enter_context(tc.tile_pool(name="nrm", bufs=4))

            for pair in range(4):
                for ib2 in range(2):
                    ibase = ib2 * 1024
                    jcs = list(range(8 * (ib2 + 1)))
                    ilo = {jc: max(jc * 128 - ibase, 0) for jc in jcs}
                    # first/last jc touching each 512-wide psum bank
                    bank_jcs = {nb: [jc for jc in jcs if ilo[jc] < nb * 512 + 512]
                                for nb in range(2)}

                    attn_ps = {}
                    exp_tiles = {}
                    for hh in range(2):
                        h = pair * 2 + hh
                        attn_ps[h] = at_pool.tile([33, 1024], F32, name=f"attn_ps{hh}", tag=f"attn_ps{hh}")

                    for jc in jcs:
                        lo = ilo[jc]
                        for hh in range(2):
                            h = pair * 2 + hh
                            dpart = slice(hh * 64, hh * 64 + 64)
                            s_ps = sc_pool.tile([128, 1024], F32, name=f"s{hh}", tag=f"s{hh}")
                            for nb in range(2):
                                a = max(lo, nb * 512)
                                bb = nb * 512 + 512
                                if a >= bb:
                                    continue
                                nc.tensor.matmul(
                                    s_ps[:, a:bb],
                                    k_sb[pair][dpart, jc * 128:(jc + 1) * 128],
                                    q_sb[pair][dpart, ibase + a:ibase + bb],
                                    start=True, stop=True)
                            et = exp_pool.tile([128, 1024], F32R, name=f"e{hh}", tag=f"e{hh}")
                            nc.scalar.activation(out=et[:, lo:1024], in_=s_ps[:, lo:1024],
                                                 func=AF.Exp, scale=SCALE)
                            if jc >= 8 * ib2:  # diagonal block: triangular mask
                                nc.vector.tensor_mul(
                                    out=et[:, lo:lo + 128],
                                    in0=et[:, lo:lo + 128].bitcast(F32),
                                    in1=tri_sb)
                            for nb in range(2):
                                a = max(lo, nb * 512)
                                bb = nb * 512 + 512
                                if a >= bb:
                                    continue
                                nc.tensor.matmul(
                                    attn_ps[h][:, a:bb],
                                    v_sb[jc][:, h * 33:(h + 1) * 33],
                                    et[:, a:bb],
                                    start=(jc == bank_jcs[nb][0]),
                                    stop=(jc == bank_jcs[nb][-1]))

                    # normalize: rows 0..31 /= row 32
                    for hh in range(2):
                        h = pair * 2 + hh
                        t, roff = h // 4, (h % 4) * 32
                        rec = nrm_pool.tile([1, 1024], F32, name=f"rec{hh}", tag=f"rec{hh}")
                        nc.vector.reciprocal(out=rec, in_=attn_ps[h][32:33, :])
                        rec_bc = nrm_pool.tile([32, 1024], F32, name=f"recbc{hh}", tag=f"recbc{hh}")
                        nc.gpsimd.partition_broadcast(rec_bc, rec)
                        nc.vector.tensor_mul(
                            out=attn_sb[t][roff:roff + 32, ibase:ibase + 1024],
                            in0=attn_ps[h][0:32, :],
                            in1=rec_bc)

        # =========== phase 3: output projection ===========
        with ExitStack() as octx:
            wo_pool = octx.enter_context(tc.tile_pool(name="wo", bufs=1))
            ob_pool = octx.enter_context(tc.tile_pool(name="ob", bufs=4))
            op_pool = octx.enter_context(tc.tile_pool(name="ops", bufs=4, space="PSUM"))

            wout_sb = []
            for t in range(2):
                w = wo_pool.tile([128, E], F32R, name=f"wo{t}", tag=f"wo{t}")
                nc.sync.dma_start(out=w, in_=wout[t * 128:(t + 1) * 128, :])
                wout_sb.append(w)
            bout_sb = wo_pool.tile([1, E], F32R, name="bo", tag="bo")
            nc.sync.dma_start(out=bout_sb, in_=bout)

            for it in range(16):
                for eb in range(2):
                    sl = slice(eb * 512, eb * 512 + 512)
                    ps = op_pool.tile([128, 512], F32, name="ops", tag="ops")
                    for t in range(2):
                        nc.tensor.matmul(ps, attn_sb[t][:, it * 128:(it + 1) * 128],
                                         wout_sb[t][:, sl],
                                         start=(t == 0), stop=False)
                    nc.tensor.matmul(ps, ones[:, 0:128], bout_sb[:, sl],
                                     start=False, stop=True)
                    ot = ob_pool.tile([128, 512], F32, name="ot", tag="ot")
                    nc.vector.tensor_copy(out=ot, in_=ps)
                    nc.sync.dma_start(out=out[it * 128:(it + 1) * 128, sl], in_=ot)

    nc.compile()
    return nc


def _get_program():
    if "nc" not in _PROGRAM_CACHE:
        _PROGRAM_CACHE["nc"] = _build_program()
    return _PROGRAM_CACHE["nc"]


def kernel(hidden_states, q_weight, q_bias, k_weight, k_bias,
           low_rank_value_weight, low_rank_value_bias,
           low_rank_output_weight, output_bias, _want_trace=False):
    hidden_states = np.asarray(hidden_states, dtype=np.float32)
    q_weight = np.asarray(q_weight, dtype=np.float32)
    q_bias = np.asarray(q_bias, dtype=np.float32)
    k_weight = np.asarray(k_weight, dtype=np.float32)
    k_bias = np.asarray(k_bias, dtype=np.float32)
    wv_full = np.asarray(low_rank_value_weight, dtype=np.float32)
    bv_full = np.asarray(low_rank_value_bias, dtype=np.float32)
    wout_full = np.asarray(low_rank_output_weight, dtype=np.float32)
    output_bias = np.asarray(output_bias, dtype=np.float32)

    tri = np.triu(np.ones((128, 128), dtype=np.float32))  # tri[p, f] = 1 if p <= f

    in_maps = []
    for c in range(N_CORES):
        b, g = c // 2, c % 2
        hs_t = np.ascontiguousarray(hidden_states[b].T)          # [E, S]
        cols = slice(g * 512, (g + 1) * 512)                     # q/k head cols
        vcols = slice(g * 256, (g + 1) * 256)                    # v head cols
        # v_aug weights: per head 32 cols + a zero col (bias row carries the 1)
        wv_aug = np.zeros((E, HG * 33), dtype=np.float32)
        bv_aug = np.zeros((1, HG * 33), dtype=np.float32)
        wv_g = wv_full[:, vcols].reshape(E, HG, R)
        bv_g = bv_full[vcols].reshape(HG, R)
        for h in range(HG):
            wv_aug[:, h * 33:h * 33 + 32] = wv_g[:, h, :]
            bv_aug[0, h * 33:h * 33 + 32] = bv_g[h]
            bv_aug[0, h * 33 + 32] = 1.0
        in_maps.append({
            "hs_t": hs_t,
            "wq": np.ascontiguousarray(q_weight[:, cols]),
            "wk": np.ascontiguousarray(k_weight[:, cols]),
            "bq": np.ascontiguousarray(q_bias[cols])[None, :],
            "bk": np.ascontiguousarray(k_bias[cols])[None, :],
            "wv": wv_aug,
            "bv": bv_aug,
            "wout": np.ascontiguousarray(wout_full[vcols, :]),
            "bout": (output_bias if g == 0
                     else np.zeros_like(output_bias))[None, :],
            "tri": tri,
        })

    nc = _get_program()
    res = run_bass_kernel_spmd(nc, in_maps, list(range(N_CORES)),
                               trace=_want_trace)
    out = np.empty((B, S, E), dtype=np.float32)
    for b in range(B):
        out[b] = res.results[2 * b]["out"] + res.results[2 * b + 1]["out"]
    if _want_trace:
        return out, res
    return out


# revision 9
# speedup vs baseline: 1.0088x; 1.0088x over previous
"""CompressedGPT2Attention on 8 TRN2 NeuronCores.

Sharding: core c = (batch b = c // 2, head-group g = c % 2) — data parallel on
B=4, tensor parallel over 16 heads (8 per group). Each core computes a partial
output [S, E] (its head-group's contribution, + output_bias on g==0 cores);
host sums the two partials per batch.

Per-core pipeline (matmul operands float32r = raw fp32 bits, PE rounds):
  hs_t [E, S] (host-transposed)
  v_aug    = hs @ Wv_aug  [S, 8*33] (per head 32 v-cols + ones col -> denominator)
  q^T, k^T = W^T @ hs_t   [512, S]  (2 heads per 128-partition tile; bias via
                                     ACT/DVE per-partition add on psum->sbuf copy)
  per head, scores^T[j, i] = k_h^T.T @ q_h^T (K=64); causal mask on diagonal
    blocks = extra PE accumulate of identity.T @ (-1e4 lower-triangle) so exp
    underflows to exactly 0
  exp via ScalarE (scale=1/8 folded), f32r out
  attn_aug^T[33, i] += v_aug_jc^T.T @ exp_jc  (PSUM accum over j-chunks)
  stage attn psum -> SBUF, normalize rows 0..31 by 1/row32 (DVE recip,
    gpsimd partition_broadcast, DVE mult)
  out[i, e] = attn^T.T @ Wout + bias (bias via gpsimd broadcast + DVE add)
"""

import numpy as np
from contextlib import ExitStack

import concourse.bass as bass
import concourse.bacc as bacc
import concourse.tile as tile
import concourse.mybir as mybir
from concourse.bass_utils import run_bass_kernel_spmd

F32 = mybir.dt.float32
F32R = mybir.dt.float32r
AF = mybir.ActivationFunctionType

B, S, E = 4, 2048, 1024
H, HD, R = 16, 64, 32
HG = 8                # heads per core
N_CORES = 8
SCALE = 1.0 / 8.0     # 1/sqrt(HD)
NEG = -1.0e4

_PROGRAM_CACHE = {}


def _build_program():
    nc = bacc.Bacc("TRN2", target_bir_lowering=False, debug=False,
                   num_devices=N_CORES)

    hs_t = nc.dram_tensor("hs_t", [E, S], F32R, kind="ExternalInput").ap()
    wq = nc.dram_tensor("wq", [E, 512], F32R, kind="ExternalInput").ap()
    wk = nc.dram_tensor("wk", [E, 512], F32R, kind="ExternalInput").ap()
    bqt = nc.dram_tensor("bqt", [128, 4], F32, kind="ExternalInput").ap()
    bkt = nc.dram_tensor("bkt", [128, 4], F32, kind="ExternalInput").ap()
    wv = nc.dram_tensor("wv", [E, HG * 33], F32R, kind="ExternalInput").ap()
    bv = nc.dram_tensor("bv", [1, HG * 33], F32, kind="ExternalInput").ap()
    wout = nc.dram_tensor("wout", [256, E], F32R, kind="ExternalInput").ap()
    bout = nc.dram_tensor("bout", [1, E], F32, kind="ExternalInput").ap()
    id128 = nc.dram_tensor("id128", [128, 128], F32R, kind="ExternalInput").ap()
    cmask = nc.dram_tensor("cmask", [128, 128], F32R, kind="ExternalInput").ap()
    out = nc.dram_tensor("out", [S, E], F32, kind="ExternalOutput").ap()

    with tile.TileContext(nc) as tc, ExitStack() as ctx:
        persist = ctx.enter_context(tc.tile_pool(name="persist", bufs=1))

        # ---- persistent activations / constants ----
        q_sb = [persist.tile([128, S], F32R, name=f"q{m}", tag=f"q{m}") for m in range(4)]
        k_sb = [persist.tile([128, S], F32R, name=f"k{m}", tag=f"k{m}") for m in range(4)]
        v_sb = [persist.tile([128, HG * 33], F32R, name=f"v{sc}", tag=f"v{sc}")
                for sc in range(16)]
        attn_sb = [persist.tile([128, S], F32R, name=f"attn{t}", tag=f"attn{t}")
                   for t in range(2)]
        id_sb = persist.tile([128, 128], F32R, name="id", tag="id")
        cm_sb = persist.tile([128, 128], F32R, name="cm", tag="cm")

        # =========== phase 1: projections ===========
        with ExitStack() as pctx:
            hs_pool = pctx.enter_context(tc.tile_pool(name="hs", bufs=1))
            w_pool = pctx.enter_context(tc.tile_pool(name="w", bufs=1))
            b_pool = pctx.enter_context(tc.tile_pool(name="b", bufs=1))
            pp = pctx.enter_context(tc.tile_pool(name="pp", bufs=4, space="PSUM"))

            # DMA order: small weights for the first groups, then hs chunks,
            # then the rest. Dynamic HW queues round-robin so these overlap.
            w_sb = {0: [], 1: []}
            wv_sb = []
            for ec in range(8):
                t = w_pool.tile([128, HG * 33], F32R, name=f"wv{ec}", tag=f"wv{ec}")
                nc.sync.dma_start(out=t, in_=wv[ec * 128:(ec + 1) * 128, :])
                wv_sb.append(t)
            bv_sb = b_pool.tile([1, HG * 33], F32, name="bv", tag="bv")
            nc.sync.dma_start(out=bv_sb, in_=bv)
            bv_bc = b_pool.tile([128, HG * 33], F32, name="bv_bc", tag="bv_bc")
            nc.gpsimd.partition_broadcast(bv_bc, bv_sb)
            nc.sync.dma_start(out=id_sb, in_=id128)
            nc.sync.dma_start(out=cm_sb, in_=cmask)

            hs_sb = []
            for ec in range(8):
                t = hs_pool.tile([128, S], F32R, name=f"hs{ec}", tag=f"hs{ec}")
                nc.sync.dma_start(out=t[:, 0:1024], in_=hs_t[ec * 128:(ec + 1) * 128, 0:1024])
                nc.sync.dma_start(out=t[:, 1024:2048], in_=hs_t[ec * 128:(ec + 1) * 128, 1024:2048])
                hs_sb.append(t)

            for which, w_dram in ((0, wq), (1, wk)):
                for ec in range(8):
                    t = w_pool.tile([128, 512], F32R, name=f"w{which}_{ec}",
                                    tag=f"w{which}_{ec}")
                    nc.sync.dma_start(out=t, in_=w_dram[ec * 128:(ec + 1) * 128, :])
                    w_sb[which].append(t)
            bqt_sb = b_pool.tile([128, 4], F32, name="bqt", tag="bqt")
            nc.sync.dma_start(out=bqt_sb, in_=bqt)
            bkt_sb = b_pool.tile([128, 4], F32, name="bkt", tag="bkt")
            nc.sync.dma_start(out=bkt_sb, in_=bkt)

            # v_aug first (small, unblocks attention earliest):
            # psum[s 128, 264] = sum_e hs_t[e, s].T @ wv[e, :]
            for sc in range(16):
                ps = pp.tile([128, HG * 33], F32, name="vproj", tag="vproj")
                for ec in range(8):
                    nc.tensor.matmul(ps, hs_sb[ec][:, sc * 128:(sc + 1) * 128],
                                     wv_sb[ec], start=(ec == 0), stop=(ec == 7))
                nc.vector.tensor_add(out=v_sb[sc], in0=ps, in1=bv_bc)

            # q^T / k^T per m-tile (pair m unblocks as soon as q[m], k[m] done):
            # psum[col 128, s 512] = sum_e w[e, col].T @ hs_t[e, s]
            for m in range(4):
                for which, bias_sb, dst in ((0, bqt_sb, q_sb), (1, bkt_sb, k_sb)):
                    for nb in range(4):
                        ps = pp.tile([128, 512], F32, name="proj", tag="proj")
                        sl = slice(nb * 512, nb * 512 + 512)
                        for ec in range(8):
                            nc.tensor.matmul(ps, w_sb[which][ec][:, m * 128:(m + 1) * 128],
                                             hs_sb[ec][:, sl],
                                             start=(ec == 0), stop=(ec == 7))
                        # psum->sbuf with per-partition bias add; alternate
                        # ACT/DVE to balance engines
                        if nb % 2 == 0:
                            nc.scalar.activation(out=dst[m][:, sl], in_=ps,
                                                 func=AF.Identity,
                                                 bias=bias_sb[:, m:m + 1], scale=1.0)
                        else:
                            nc.vector.tensor_scalar_add(out=dst[m][:, sl], in0=ps,
                                                        scalar1=bias_sb[:, m:m + 1])

        # =========== phase 2: attention ===========
        with ExitStack() as actx:
            sc_pool = actx.enter_context(tc.tile_pool(name="scps", bufs=1, space="PSUM"))
            at_pool = actx.enter_context(tc.tile_pool(name="atps", bufs=1, space="PSUM"))
            exp_pool = actx.enter_context(tc.tile_pool(name="exp", bufs=4))
            nrm_pool = actx.enter_context(tc.tile_pool(name="nrm", bufs=2))

            for pair in range(4):
                for ib2 in range(2):
                    ibase = ib2 * 1024
                    jcs = list(range(8 * (ib2 + 1)))
                    ilo = {jc: max(jc * 128 - ibase, 0) for jc in jcs}
                    bank_jcs = {nb: [jc for jc in jcs if ilo[jc] < nb * 512 + 512]
                                for nb in range(2)}

                    attn_ps = {}
                    for hh in range(2):
                        h = pair * 2 + hh
                        attn_ps[h] = at_pool.tile([33, 1024], F32,
                                                  name=f"attn_ps{hh}", tag=f"attn_ps{hh}")

                    for jc in jcs:
                        lo = ilo[jc]
                        s_ps = {}
                        # scores for both heads first (adjacent MMs on distinct
                        # 64-row groups can overlap in the PE array)
                        diag = jc >= 8 * ib2
                        for hh in range(2):
                            h = pair * 2 + hh
                            dpart = slice(hh * 64, hh * 64 + 64)
                            sp = sc_pool.tile([128, 1024], F32, name=f"s{hh}", tag=f"s{hh}")
                            for nb in range(2):
                                a = max(lo, nb * 512)
                                bb = nb * 512 + 512
                                if a >= bb:
                                    continue
                                mask_here = diag and a == lo
                                nc.tensor.matmul(
                                    sp[:, a:bb],
                                    k_sb[pair][dpart, jc * 128:(jc + 1) * 128],
                                    q_sb[pair][dpart, ibase + a:ibase + bb],
                                    start=True, stop=not mask_here)
                                if mask_here:
                                    # += id.T @ cmask: -1e4 below diagonal
                                    nc.tensor.matmul(sp[:, lo:lo + 128], id_sb, cm_sb,
                                                     start=False, stop=True)
                            s_ps[hh] = sp
                        for hh in range(2):
                            h = pair * 2 + hh
                            et = exp_pool.tile([128, 1024], F32R, name=f"e{hh}", tag=f"e{hh}")
                            nc.scalar.activation(out=et[:, lo:1024],
                                                 in_=s_ps[hh][:, lo:1024],
                                                 func=AF.Exp, scale=SCALE)
                            for nb in range(2):
                                a = max(lo, nb * 512)
                                bb = nb * 512 + 512
                                if a >= bb:
                                    continue
                                nc.tensor.matmul(
                                    attn_ps[h][:, a:bb],
                                    v_sb[jc][:, h * 33:(h + 1) * 33],
                                    et[:, a:bb],
                                    start=(jc == bank_jcs[nb][0]),
                                    stop=(jc == bank_jcs[nb][-1]))

                    # stage to SBUF (frees psum), then normalize rows /= row 32
                    for hh in range(2):
                        h = pair * 2 + hh
                        t, roff = h // 4, (h % 4) * 32
                        araw = nrm_pool.tile([33, 1024], F32, name=f"araw{hh}", tag=f"araw{hh}")
                        nc.vector.tensor_copy(out=araw, in_=attn_ps[h])
                        rec = nrm_pool.tile([1, 1024], F32, name=f"rec{hh}", tag=f"rec{hh}")
                        nc.vector.reciprocal(out=rec, in_=araw[32:33, :])
                        rec_bc = nrm_pool.tile([32, 1024], F32, name=f"recbc{hh}",
                                               tag=f"recbc{hh}")
                        nc.gpsimd.partition_broadcast(rec_bc, rec)
                        nc.vector.tensor_mul(
                            out=attn_sb[t][roff:roff + 32, ibase:ibase + 1024],
                            in0=araw[0:32, :],
                            in1=rec_bc)

        # =========== phase 3: output projection ===========
        with ExitStack() as octx:
            wo_pool = octx.enter_context(tc.tile_pool(name="wo", bufs=1))
            ob_pool = octx.enter_context(tc.tile_pool(name="ob", bufs=4))
            op_pool = octx.enter_context(tc.tile_pool(name="ops", bufs=4, space="PSUM"))

            wout_sb = []
            for t in range(2):
                w = wo_pool.tile([128, E], F32R, name=f"wo{t}", tag=f"wo{t}")
                nc.sync.dma_start(out=w, in_=wout[t * 128:(t + 1) * 128, :])
                wout_sb.append(w)
            bout_sb = wo_pool.tile([1, E], F32, name="bo", tag="bo")
            nc.sync.dma_start(out=bout_sb, in_=bout)
            bout_bc = wo_pool.tile([128, E], F32, name="bo_bc", tag="bo_bc")
            nc.gpsimd.partition_broadcast(bout_bc, bout_sb)

            for it in range(16):
                for eb in range(2):
                    sl = slice(eb * 512, eb * 512 + 512)
                    ps = op_pool.tile([128, 512], F32, name="ops", tag="ops")
                    for t in range(2):
                        nc.tensor.matmul(ps, attn_sb[t][:, it * 128:(it + 1) * 128],
                                         wout_sb[t][:, sl],
                                         start=(t == 0), stop=(t == 1))
                    ot = ob_pool.tile([128, 512], F32, name="ot", tag="ot")
                    nc.vector.tensor_add(out=ot, in0=ps, in1=bout_bc[:, sl])
                    nc.sync.dma_start(out=out[it * 128:(it + 1) * 128, sl], in_=ot)

    nc.compile()
    return nc


def _get_program():
    if "nc" not in _PROGRAM_CACHE:
        _PROGRAM_CACHE["nc"] = _build_program()
    return _PROGRAM_CACHE["nc"]


def kernel(hidden_states, q_weight, q_bias, k_weight, k_bias,
           low_rank_value_weight, low_rank_value_bias,
           low_rank_output_weight, output_bias):
    hidden_states = np.asarray(hidden_states, dtype=np.float32)
    q_weight = np.asarray(q_weight, dtype=np.float32)
    q_bias = np.asarray(q_bias, dtype=np.float32)
    k_weight = np.asarray(k_weight, dtype=np.float32)
    k_bias = np.asarray(k_bias, dtype=np.float32)
    wv_full = np.asarray(low_rank_value_weight, dtype=np.float32)
    bv_full = np.asarray(low_rank_value_bias, dtype=np.float32)
    wout_full = np.asarray(low_rank_output_weight, dtype=np.float32)
    output_bias = np.asarray(output_bias, dtype=np.float32)

    id128 = np.eye(128, dtype=np.float32)
    cmask = np.where(np.arange(128)[None, :] < np.arange(128)[:, None],
                     np.float32(NEG), np.float32(0.0))  # -1e4 where i < j

    in_maps = []
    for c in range(N_CORES):
        b, g = c // 2, c % 2
        hs_t = np.ascontiguousarray(hidden_states[b].T)          # [E, S]
        cols = slice(g * 512, (g + 1) * 512)                     # q/k head cols
        vcols = slice(g * 256, (g + 1) * 256)                    # v head cols
        wv_aug = np.zeros((E, HG * 33), dtype=np.float32)
        bv_aug = np.zeros((1, HG * 33), dtype=np.float32)
        wv_g = wv_full[:, vcols].reshape(E, HG, R)
        bv_g = bv_full[vcols].reshape(HG, R)
        for h in range(HG):
            wv_aug[:, h * 33:h * 33 + 32] = wv_g[:, h, :]
            bv_aug[0, h * 33:h * 33 + 32] = bv_g[h]
            bv_aug[0, h * 33 + 32] = 1.0
        in_maps.append({
            "hs_t": hs_t,
            "wq": np.ascontiguousarray(q_weight[:, cols]),
            "wk": np.ascontiguousarray(k_weight[:, cols]),
            "bqt": np.ascontiguousarray(q_bias[cols].reshape(4, 128).T),
            "bkt": np.ascontiguousarray(k_bias[cols].reshape(4, 128).T),
            "wv": wv_aug,
            "bv": bv_aug,
            "wout": np.ascontiguousarray(wout_full[vcols, :]),
            "bout": (output_bias if g == 0
                     else np.zeros_like(output_bias))[None, :],
            "id128": id128,
            "cmask": cmask,
        })

    nc = _get_program()
    res = run_bass_kernel_spmd(nc, in_maps, list(range(N_CORES)))
    out = np.empty((B, S, E), dtype=np.float32)
    for b in range(B):
        out[b] = res.results[2 * b]["out"] + res.results[2 * b + 1]["out"]
    return out
